# revision 11
# baseline (speedup 1.0000x reference)
"""BitLinear 1.58 Trainium2 Bass kernel — v5.

2D sharding: 4 token-quarters x 2 weight-halves. Each core:
x_shard [2048, 4096], w_half [2048, 4096] -> outT [2048 o, 2048 m] f16
(host transposes each core's block and reassembles the full f32 output).

Structure:
  - Ternary weight = matmul STATIONARY operand (fp8e4 {-2,0,2}, 16
    resident 0.5MB unit tiles, LDWEIGHTS hidden behind N=512 f16 moving
    streams at 1 col/cycle ~216ns/MM).
  - Output comes out transposed [o, m], so the row scale bp = 0.5*s is
    per-PARTITION: evictions are Act Copy(ps*bp) -> f16, no DRAM scale
    round-trip.
  - Block order (mq, unit-half): all four m-quads sweep units 0-7
    first, then units 8-15 — the quantized x^T m-quad tiles (4MB f16)
    are SPILLED to DRAM after phase L and RELOADED for phase H, buying
    a 2-deep ring in SBUF while weight-unit demand spreads over ~400us.
  - Weight unit chain: |w| row-mean two-stage on DVE, 2x Sign on Act,
    subtract on DVE, bf16 transpose on SP hwdge (tscr ring 2), fp8
    convert on Act.
  - x quant per 128-token block (halves): absmax on DVE, tiny scales on
    Pool, q = x*scale -> f16 on Pool, magic round f16 on DVE, dequant
    f16 on Pool, f16 transpose into the m-quad tile.
"""
import sys

sys.path.insert(0, "/opt/trn_rl_repo")

import numpy as np

B, S, D_IN, D_OUT = 4, 2048, 4096, 4096
N_CORES = 8
TQ = 4
WQ = 2
M_TOT = B * S
M_C = M_TOT // TQ               # 2048 tokens per core
O_C = D_OUT // WQ               # 2048 out cols per core
K = D_IN

P = 128
G = 64
MW = 512                        # moving m-width (one psum bank)
MAGIC16 = float(1.5 * 2.0 ** 10)
EPS = 1e-5
QMAX = 127.0
INV_QMAX = float(np.float32(1.0 / 127.0))

MB = M_C // P                   # 16 token blocks
NU = O_C // P                   # 16 weight units
NMQ = M_C // MW                 # 4 mquads
KSUB = K // P                   # 32
KH = K // 2                     # 2048
NGH = KH // G                   # 32 x-quant groups per half
NGW = KH // G                   # 32 w-sum groups per half

_cache = {}


def _build():
    import concourse.tile as tile
    from concourse import bacc, mybir

    f32 = mybir.dt.float32
    f16 = mybir.dt.float16
    bf16 = mybir.dt.bfloat16
    fp8 = mybir.dt.float8e4
    Alu = mybir.AluOpType
    Act = mybir.ActivationFunctionType
    Ax = mybir.AxisListType

    nc = bacc.Bacc("TRN2", target_bir_lowering=False, num_devices=1)
    x = nc.dram_tensor("x", [M_C, K], f32, kind="ExternalInput")
    w = nc.dram_tensor("w", [O_C, K], f32, kind="ExternalInput")
    outT = nc.dram_tensor("outT", [O_C, M_C], f16, kind="ExternalOutput")
    xspill = nc.dram_tensor("xspill", [NMQ * P, KSUB * MW], f16,
                            kind="Internal")

    xap, wap, oap, xsap = x.ap(), w.ap(), outT.ap(), xspill.ap()

    with tile.TileContext(nc) as tc:
        with (
            tc.tile_pool(name="tt", bufs=NU) as tt_pool,
            tc.tile_pool(name="xq", bufs=2) as xq_pool,
            tc.tile_pool(name="wst", bufs=2) as wstage,
            tc.tile_pool(name="xst", bufs=2) as xstage,
            tc.tile_pool(name="xq16", bufs=2) as xq16_pool,
            tc.tile_pool(name="sg", bufs=3) as sg_pool,
            tc.tile_pool(name="tscr", bufs=2) as tscr_pool,
            tc.tile_pool(name="evq", bufs=4) as ev_pool,
            tc.tile_pool(name="bp", bufs=NU) as bp_pool,
            tc.tile_pool(name="small", bufs=4) as small,
            tc.tile_pool(name="ps", bufs=6, space="PSUM") as ps_pool,
        ):
            tt_tiles = {}
            bp_tiles = {}
            xq_tiles = {}
            w_stage = {}

            def w_load(u):
                ts = []
                for h in range(2):
                    wst = wstage.tile([P, KH], f32, tag="wst",
                                      name=f"wst{u}_{h}")
                    nc.sync.dma_start(
                        wst[:], wap[u * P:(u + 1) * P,
                                    h * KH:(h + 1) * KH])
                    ts.append(wst)
                w_stage[u] = ts

            def w_chain(u):
                whs = w_stage.pop(u)
                sh = small.tile([P, 2], f32, tag="sh")
                for h in range(2):
                    gs = small.tile([P, NGW], f32, tag="gs")
                    nc.vector.tensor_reduce(
                        gs[:], whs[h].rearrange("p (g e) -> p g e", e=G),
                        Ax.X, Alu.add, apply_absolute_value=True)
                    nc.vector.tensor_reduce(sh[:, h:h + 1], gs[:], Ax.X,
                                            Alu.add)
                s1 = small.tile([P, 1], f32, tag="s1")
                nc.gpsimd.tensor_tensor(s1[:], sh[:, 0:1], sh[:, 1:2],
                                        Alu.add)
                # bp = 0.5 * max(mean|row|, eps): sign bias AND the
                # eviction scale (d is in {-2,0,2})
                bp = bp_pool.tile([P, 1], f32, tag="bp", name=f"bp{u}")
                nc.gpsimd.tensor_scalar(bp[:], s1[:],
                                        float(np.float32(0.5 / K)),
                                        0.5 * EPS, Alu.mult, Alu.max)
                bp_tiles[u] = bp
                scr = tscr_pool.tile([P, KSUB, P], bf16, tag="tscr",
                                     name=f"scr{u}")
                for h in range(2):
                    sga = sg_pool.tile([P, KH], bf16, tag="sg",
                                       name=f"sga{u}_{h}")
                    nc.scalar.activation(out=sga[:], in_=whs[h][:],
                                         func=Act.Sign, bias=bp[:],
                                         scale=-1.0)
                    sgb = sg_pool.tile([P, KH], bf16, tag="sg",
                                       name=f"sgb{u}_{h}")
                    nc.scalar.activation(out=sgb[:], in_=whs[h][:],
                                         func=Act.Sign, bias=bp[:],
                                         scale=1.0)
                    nc.vector.tensor_tensor(sgb[:], sgb[:], sga[:],
                                            Alu.subtract)
                    nc.sync.dma_start_transpose(
                        scr[:, h * (KSUB // 2):(h + 1) * (KSUB // 2), :],
                        sgb[:])
                ttu = tt_pool.tile([P, KSUB, P], fp8, tag="tt",
                                   name=f"tt{u}")
                nc.scalar.activation(out=ttu[:], in_=scr[:],
                                     func=Act.Copy)
                tt_tiles[u] = ttu

            def x_quant(mb):
                mq, ml = divmod(mb, NMQ)
                if ml == 0:
                    xq_tiles[mq] = xq_pool.tile([P, KSUB, MW], f16,
                                                tag="xq", name=f"xq{mq}")
                xqt = xq_tiles[mq]
                for h in range(2):
                    xt = xstage.tile([P, KH], f32, tag="xst",
                                     name=f"xst{mb}_{h}")
                    nc.gpsimd.dma_start(
                        xt[:], xap[mb * P:(mb + 1) * P,
                                   h * KH:(h + 1) * KH])
                    xg = xt.rearrange("p (g e) -> p g e", e=G)
                    am = small.tile([P, NGH], f32, tag="am")
                    nc.vector.tensor_reduce(am[:], xg, Ax.X, Alu.max,
                                            apply_absolute_value=True)
                    am2 = small.tile([P, NGH], f32, tag="am2")
                    nc.gpsimd.tensor_scalar(am2[:], am[:], EPS, None,
                                            Alu.max)
                    rc = small.tile([P, NGH], f32, tag="rc")
                    nc.vector.reciprocal(rc[:], am2[:])
                    scale = small.tile([P, NGH], f32, tag="scale")
                    nc.gpsimd.tensor_scalar(scale[:], rc[:], QMAX, None,
                                            Alu.mult)
                    inv = small.tile([P, NGH], f16, tag="inv")
                    nc.gpsimd.tensor_scalar(inv[:], am2[:], INV_QMAX,
                                            None, Alu.mult)
                    q16 = xq16_pool.tile([P, KH], f16, tag="xq16",
                                         name=f"q16_{mb}_{h}")
                    qg = q16.rearrange("p (g e) -> p g e", e=G)
                    nc.gpsimd.tensor_tensor(
                        qg, xg,
                        scale[:, :, None].to_broadcast((P, NGH, G)),
                        Alu.mult)
                    nc.vector.tensor_scalar(q16[:], q16[:], MAGIC16,
                                            MAGIC16, Alu.add,
                                            Alu.subtract)
                    nc.gpsimd.tensor_tensor(
                        qg, qg,
                        inv[:, :, None].to_broadcast((P, NGH, G)),
                        Alu.mult)
                    nc.sync.dma_start_transpose(
                        xqt[:, h * (KSUB // 2):(h + 1) * (KSUB // 2),
                            ml * P:(ml + 1) * P], q16[:])

            def x_spill(mq):
                nc.gpsimd.dma_start(xsap[mq * P:(mq + 1) * P, :],
                                    xq_tiles[mq][:])

            def x_reload(mq):
                t = xq_pool.tile([P, KSUB, MW], f16, tag="xq",
                                 name=f"xqr{mq}")
                nc.gpsimd.dma_start(t[:], xsap[mq * P:(mq + 1) * P, :])
                xq_tiles[mq] = t

            def mm_group(mq, u):
                ps = ps_pool.tile([P, MW], f32)
                xqt = xq_tiles[mq]
                ttu = tt_tiles[u]
                for ks in range(KSUB):
                    nc.tensor.matmul(ps[:], ttu[:, ks, :], xqt[:, ks, :],
                                     start=(ks == 0),
                                     stop=(ks == KSUB - 1))
                ev = ev_pool.tile([P, MW], f16, tag="evq")
                nc.scalar.activation(out=ev[:], in_=ps[:], func=Act.Copy,
                                     scale=bp_tiles[u])
                nc.gpsimd.dma_start(
                    oap[u * P:(u + 1) * P, mq * MW:(mq + 1) * MW], ev[:])

            # ---------------- emission schedule ----------------
            w_load(0)
            w_load(1)
            x_quant(0)
            w_chain(0)
            x_quant(1)
            w_load(2)
            w_chain(1)
            x_quant(2)
            w_load(3)
            w_chain(2)
            x_quant(3)

            # blocks: every mquad over units 0-7, then (after spill /
            # reload round-trip of the xq tiles) over units 8-15
            blocks = [(mq, 0) for mq in range(NMQ)] + \
                     [(mq, 1) for mq in range(NMQ)]
            # global pending work, drained one item per matmul group;
            # force-drained for a block's xq tile / a group's tt unit
            pending = [
                ("c", 3), ("l", 4), ("c", 4), ("l", 5), ("c", 5),
                ("l", 6), ("c", 6), ("l", 7), ("c", 7),
                ("x", 4), ("x", 5), ("x", 6), ("x", 7), ("s", 0),
                ("x", 8), ("x", 9), ("x", 10), ("x", 11), ("s", 1),
                ("x", 12), ("x", 13), ("x", 14), ("x", 15), ("s", 2),
                ("l", 8), ("c", 8), ("l", 9), ("c", 9),
                ("l", 10), ("c", 10), ("l", 11), ("c", 11),
                ("s", 3), ("r", 0),
                ("l", 12), ("c", 12), ("l", 13), ("c", 13), ("r", 1),
                ("l", 14), ("c", 14), ("l", 15), ("c", 15),
                ("r", 2), ("r", 3),
            ]
            emit = {"l": w_load, "c": w_chain, "x": x_quant,
                    "s": x_spill, "r": x_reload}
            reloaded = set()
            xq_complete = {0}

            def drain1():
                kind, arg = pending.pop(0)
                emit[kind](arg)
                if kind == "r":
                    reloaded.add(arg)
                elif kind == "x" and arg % NMQ == NMQ - 1:
                    xq_complete.add(arg // NMQ)

            for mq, uh in blocks:
                ready = reloaded if uh == 1 else xq_complete
                while mq not in ready:
                    drain1()
                for u in range(uh * 8, uh * 8 + 8):
                    while u not in tt_tiles:
                        drain1()
                    mm_group(mq, u)
                    if pending:
                        drain1()
            while pending:
                drain1()

    nc.compile()
    return nc


def _get_nc():
    if "nc" not in _cache:
        _cache["nc"] = _build()
    return _cache["nc"]


def run(x, weight, trace=False):
    """Run on 8 NeuronCores; returns (full output [B,S,D_OUT], results)."""
    from concourse.bass_utils import run_bass_kernel_spmd

    x = np.ascontiguousarray(np.asarray(x, dtype=np.float32))
    w = np.ascontiguousarray(np.asarray(weight, dtype=np.float32))
    assert x.shape == (B, S, D_IN) and w.shape == (D_OUT, D_IN)
    xf = x.reshape(M_TOT, D_IN)
    nc = _get_nc()
    in_maps = []
    for c in range(N_CORES):
        tq, wq = c % TQ, c // TQ
        in_maps.append({
            "x": np.ascontiguousarray(xf[tq * M_C:(tq + 1) * M_C]),
            "w": np.ascontiguousarray(w[wq * O_C:(wq + 1) * O_C]),
        })
    res = run_bass_kernel_spmd(nc, in_maps, core_ids=list(range(N_CORES)),
                               trace=trace)
    outf = np.empty((M_TOT, D_OUT), dtype=np.float32)
    for c in range(N_CORES):
        tq, wq = c % TQ, c // TQ
        outf[tq * M_C:(tq + 1) * M_C,
             wq * O_C:(wq + 1) * O_C] = res.results[c]["outT"].T
    return outf.reshape(B, S, D_OUT), res


def kernel(x, weight):
    out, _ = run(x, weight)
    return out
